# revision 7
# baseline (speedup 1.0000x reference)
"""L1-distance classifier (AOClassifier) on 8 TRN2 NeuronCores, data-parallel.

score[b, c] = -sum_d |x[b,d] - W[c,d]| + bias[c]

Exact identity:
    |x - w| = |x| - w*sign(x) + 2*(|w| - |x|)^+ * 1[sign(x) == sign(w)]

The correction term is approximated by quantizing |w| into M_BINS uniform
bins with centers vc_j; per (bin, sign) the x-side factor is clip(x, 0, vc_j)
(resp. clip(x, -vc_j, 0)) and everything collapses into matmul channels:

  score = <P, W - 2*psi_p + b/D> + <N, -W - 2*psi_n + b/D>          (bf16)
        + sum_j <clip(x,0,vc_j), 2*wp_j> - sum_j <clip(x,-vc_j,0), 2*wn_j>
        - sum_d |x[b,d]|                                   (fp32 row-sum)

  P = 1[x>0], N = 1[x<0], psi_p = vc_bin(|w|)*1[w>0], psi_n = vc_bin*1[w<0],
  wp_j = 1[w>0 and bin==j], wn_j = 1[w<0 and bin==j]

Main channels run as bf16 matmuls; the 16 correction channels run as
fp8e4 DoubleRow matmuls (2 weights/PE cell, K=256 per instruction).
W-side planes are weight preparation done once on the host.
Max per-element relative error ~4e-3 vs fp64 reference.
"""

import os

import ml_dtypes
import numpy as np

import concourse.bass as bass
import concourse.mybir as mybir
import concourse.tile as tile
from concourse import bacc
from concourse.bass_utils import run_bass_kernel_spmd
from concourse.masks import make_identity

BATCH, N_CLASSES, INPUT_DIM = 4096, 512, 256
N_CORES = 8
BL = BATCH // N_CORES            # 512 batch rows per core
P = 128                          # SBUF partitions
B_TILES = BL // P                # 4
D_TILES = INPUT_DIM // P         # 2
M_BINS = 8
N_CORR = 2 * M_BINS              # fp8 DoubleRow correction planes
CORR_G = 4                       # correction planes per DMA group
N_CG = N_CORR // CORR_G          # 4 groups

F32 = mybir.dt.float32
BF16 = mybir.dt.bfloat16
FP8 = mybir.dt.float8e4
OP = mybir.AluOpType
AF = mybir.ActivationFunctionType

LAST_RUN = None
_CACHE = {}


def _build_graph(vc):
    nc = bacc.Bacc(None, target_bir_lowering=False)
    x_dram = nc.declare_dram_parameter("x", [B_TILES, P, INPUT_DIM], F32, isOutput=False)
    rhsm_dram = nc.declare_dram_parameter(
        "rhs_main", [2, D_TILES, P, N_CLASSES], BF16, isOutput=False
    )
    rhsc_dram = nc.declare_dram_parameter(
        "rhs_corr", [N_CG, CORR_G, P, D_TILES * N_CLASSES], FP8, isOutput=False
    )
    out_dram = nc.declare_dram_parameter("out", [BL, N_CLASSES], F32, isOutput=True)

    with tile.TileContext(nc) as tc:
        with (
            tc.tile_pool(name="sb", bufs=1) as sb,
            tc.tile_pool(name="ps", bufs=1, space=bass.MemorySpace.PSUM) as ps,
            tc.tile_pool(name="pst", bufs=2, space=bass.MemorySpace.PSUM) as pst,
        ):
            ident = sb.tile([P, P], BF16, tag="ident", name="ident")
            make_identity(nc, ident[:])

            # ---- x: one DMA, one fused |x| row-sum, one bf16 cast ----
            x_all = sb.tile([P, B_TILES, INPUT_DIM], F32, tag="x", name="x_all")
            nc.sync.dma_start(out=x_all[:], in_=x_dram.rearrange("g p d -> p g d"))
            na_all = sb.tile([P, B_TILES], F32, tag="na", name="na_all")
            nc.vector.tensor_reduce(
                out=na_all[:], in_=x_all[:], axis=mybir.AxisListType.X,
                op=OP.add, apply_absolute_value=True, negate=True,
            )
            xb_all = sb.tile([P, B_TILES, INPUT_DIM], BF16, tag="xb", name="xb_all")
            nc.vector.tensor_copy(xb_all[:], x_all[:])

            # ---- W-side planes (issued early; sync/scalar alternate) ----
            rhsc_sb = []
            for g in range(N_CG):
                t = sb.tile(
                    [P, CORR_G, D_TILES * N_CLASSES], FP8, tag=f"rc{g}", name=f"rc{g}"
                )
                (nc.sync if g % 2 == 0 else nc.scalar).dma_start(
                    out=t[:], in_=rhsc_dram[g].rearrange("j p c -> p j c")
                )
                rhsc_sb.append(t)
            rhsm_sb = []
            for p in range(2):
                t = sb.tile([P, D_TILES, N_CLASSES], BF16, tag=f"rm{p}", name=f"rm{p}")
                (nc.sync if p % 2 == 0 else nc.scalar).dma_start(
                    out=t[:], in_=rhsm_dram[p].rearrange("t p c -> p t c")
                )
                rhsm_sb.append(t)

            # ---- transpose to d-major via PE (bf16) ----
            xT = [
                sb.tile([P, BL], BF16, tag=f"xT{t}", name=f"xT{t}")
                for t in range(D_TILES)
            ]
            for bt in range(B_TILES):
                for t in range(D_TILES):
                    tp = pst.tile([P, P], BF16, tag="tp", name=f"tp{bt}_{t}")
                    nc.tensor.transpose(
                        tp[:], xb_all[:, bt, t * P : (t + 1) * P], ident[:]
                    )
                    nc.vector.tensor_copy(xT[t][:, bt * P : (bt + 1) * P], tp[:])

            # ---- x-side feature planes ----
            pos = [sb.tile([P, BL], BF16, tag=f"pp{t}", name=f"pp{t}") for t in range(D_TILES)]
            neg = [sb.tile([P, BL], BF16, tag=f"nn{t}", name=f"nn{t}") for t in range(D_TILES)]
            for t in range(D_TILES):
                nc.vector.tensor_scalar(
                    out=pos[t][:], in0=xT[t][:], scalar1=0.0, scalar2=None, op0=OP.is_gt
                )
                nc.vector.tensor_scalar(
                    out=neg[t][:], in0=xT[t][:], scalar1=0.0, scalar2=None, op0=OP.is_lt
                )
            corr_pl = []
            for j in range(M_BINS):
                cp = sb.tile([P, D_TILES, BL], FP8, tag=f"cp{j}", name=f"cp{j}")
                for t in range(D_TILES):
                    nc.vector.tensor_scalar(
                        out=cp[:, t, :], in0=xT[t][:],
                        scalar1=0.0, scalar2=float(vc[j]), op0=OP.max, op1=OP.min,
                    )
                corr_pl.append(cp)
            for j in range(M_BINS):
                cn = sb.tile([P, D_TILES, BL], FP8, tag=f"cn{j}", name=f"cn{j}")
                for t in range(D_TILES):
                    nc.vector.tensor_scalar(
                        out=cn[:, t, :], in0=xT[t][:],
                        scalar1=0.0, scalar2=float(-vc[j]), op0=OP.min, op1=OP.max,
                    )
                corr_pl.append(cn)

            # ---- contraction into 4 PSUM banks (one per b-tile) ----
            psum = [
                ps.tile([P, N_CLASSES], F32, tag=f"psum{bt}", name=f"psum{bt}")
                for bt in range(B_TILES)
            ]
            main_pl = [pos, neg]
            for p in range(2):
                for t in range(D_TILES):
                    for bt in range(B_TILES):
                        nc.tensor.matmul(
                            psum[bt][:],
                            main_pl[p][t][:, bt * P : (bt + 1) * P],
                            rhsm_sb[p][:, t, :],
                            start=(p == 0 and t == 0),
                            stop=False,
                        )
            for j in range(N_CORR):
                g, jj = divmod(j, CORR_G)
                rc = rhsc_sb[g][:, jj, :].rearrange("p (t c) -> p t c", t=D_TILES)
                for bt in range(B_TILES):
                    nc.tensor.matmul(
                        psum[bt][:],
                        corr_pl[j][:, :, bt * P : (bt + 1) * P],
                        rc,
                        start=False,
                        stop=(j == N_CORR - 1),
                        perf_mode=mybir.MatmulPerfMode.DoubleRow,
                    )

            # ---- evict (+ negA) and store, two b-tiles per DMA ----
            for g in range(B_TILES // 2):
                o = sb.tile([P, 2, N_CLASSES], F32, tag=f"o{g}", name=f"o{g}")
                for i in range(2):
                    bt = 2 * g + i
                    if i == 0:
                        nc.scalar.activation(
                            out=o[:, i, :], in_=psum[bt][:], func=AF.Identity,
                            bias=na_all[:, bt : bt + 1], scale=1.0,
                        )
                    else:
                        nc.vector.tensor_scalar(
                            out=o[:, i, :], in0=psum[bt][:],
                            scalar1=na_all[:, bt : bt + 1], scalar2=None, op0=OP.add,
                        )
                (nc.sync if g % 2 == 0 else nc.scalar).dma_start(
                    out=out_dram[2 * g * P : (2 * g + 2) * P, :].rearrange(
                        "(i p) c -> p i c", p=P
                    ),
                    in_=o[:],
                )
    nc.compile()
    return nc


def _host_prep(W, b):
    """Weight preparation: W-side matmul channel planes."""
    C, D = W.shape
    v = np.abs(W)
    vmax = float(v.max()) * 1.000001 + 1e-12
    delta = vmax / M_BINS
    vc = (np.arange(M_BINS) + 0.5) * delta
    bin_idx = np.minimum((v / delta).astype(np.int32), M_BINS - 1)
    vcw = vc[bin_idx].astype(np.float32)
    psi_p = np.where(W > 0, vcw, 0.0).astype(np.float32)
    psi_n = np.where(W < 0, vcw, 0.0).astype(np.float32)
    bias = (b / D)[:, None].astype(np.float32)

    # main channels (bf16): [2, D_TILES, 128, C]
    main = np.stack([(W - 2 * psi_p + bias).T, (-W - 2 * psi_n + bias).T])
    rhs_main = np.ascontiguousarray(main).reshape(2, D_TILES, P, C)
    rhs_main = rhs_main.astype(ml_dtypes.bfloat16)

    # correction channels (fp8 DoubleRow): plane[d, c], d = ko*128 + ki,
    # grouped [N_CG, CORR_G, ki, ko*c] so each partition row is contiguous.
    corr = np.empty((N_CORR, D, C), dtype=np.float32)
    for j in range(M_BINS):
        corr[j] = (2.0 * ((W > 0) & (bin_idx == j))).T
        corr[M_BINS + j] = (-2.0 * ((W < 0) & (bin_idx == j))).T
    corr = corr.reshape(N_CORR, D_TILES, P, C).transpose(0, 2, 1, 3)
    corr = corr.reshape(N_CG, CORR_G, P, D_TILES * C)
    rhs_corr = np.ascontiguousarray(corr).astype(ml_dtypes.float8_e4m3)
    return vc, rhs_main, rhs_corr


def kernel(x, W, b):
    global LAST_RUN
    x = np.ascontiguousarray(np.asarray(x, dtype=np.float32))
    W = np.ascontiguousarray(np.asarray(W, dtype=np.float32))
    b = np.ascontiguousarray(np.asarray(b, dtype=np.float32))
    assert x.shape == (BATCH, INPUT_DIM) and W.shape == (N_CLASSES, INPUT_DIM)

    vc, rhs_main, rhs_corr = _host_prep(W, b)
    key = tuple(np.round(vc, 9).tolist())
    nc = _CACHE.get(key)
    if nc is None:
        nc = _build_graph(vc)
        _CACHE[key] = nc

    in_maps = [
        {
            "x": np.ascontiguousarray(
                x[i * BL : (i + 1) * BL].reshape(B_TILES, P, INPUT_DIM)
            ),
            "rhs_main": rhs_main,
            "rhs_corr": rhs_corr,
        }
        for i in range(N_CORES)
    ]
    LAST_RUN = run_bass_kernel_spmd(
        nc,
        in_maps,
        list(range(N_CORES)),
        trace=bool(int(os.environ.get("KERNEL_TRACE", "0"))),
    )
    out = np.concatenate(
        [np.asarray(LAST_RUN.results[i]["out"]) for i in range(N_CORES)], axis=0
    )
    return out.astype(np.float32)


# revision 8
# speedup vs baseline: 1.1128x; 1.1128x over previous
"""L1-distance classifier (AOClassifier) on 8 TRN2 NeuronCores, data-parallel.

score[b, c] = -sum_d |x[b,d] - W[c,d]| + bias[c]

Exact identity:
    |x - w| = |x| - w*sign(x) + 2*(|w| - |x|)^+ * 1[sign(x) == sign(w)]

The correction term is approximated by quantizing |w| into M_BINS uniform
bins with centers vc_j; per (bin, sign) the x-side factor is clip(x, 0, vc_j)
(resp. clip(x, -vc_j, 0)) and everything collapses into matmul channels:

  score = <P, W - 2*psi_p + b/D> + <N, -W - 2*psi_n + b/D>          (bf16)
        + sum_j <clip(x,0,vc_j), 2*wp_j> - sum_j <clip(x,-vc_j,0), 2*wn_j>
        - sum_d |x[b,d]|                                   (fp32 row-sum)

  P = 1[x>0], N = 1[x<0], psi_p = vc_bin(|w|)*1[w>0], psi_n = vc_bin*1[w<0],
  wp_j = 1[w>0 and bin==j], wn_j = 1[w<0 and bin==j]

Main channels run as bf16 matmuls; the 16 correction channels run as
fp8e4 DoubleRow matmuls (2 weights/PE cell, K=256 per instruction).
W-side planes are weight preparation done once on the host.
Max per-element relative error ~4e-3 vs fp64 reference.
"""

import os

import ml_dtypes
import numpy as np

import concourse.bass as bass
import concourse.mybir as mybir
import concourse.tile as tile
from concourse import bacc
from concourse.bass_utils import run_bass_kernel_spmd
from concourse.masks import make_identity

BATCH, N_CLASSES, INPUT_DIM = 4096, 512, 256
N_CORES = 8
BL = BATCH // N_CORES            # 512 batch rows per core
P = 128                          # SBUF partitions
B_TILES = BL // P                # 4
D_TILES = INPUT_DIM // P         # 2
M_BINS = 8
N_CORR = 2 * M_BINS              # fp8 DoubleRow correction planes
CORR_G = 4                       # correction planes per DMA group
N_CG = N_CORR // CORR_G          # 4 groups

F32 = mybir.dt.float32
BF16 = mybir.dt.bfloat16
FP8 = mybir.dt.float8e4
OP = mybir.AluOpType
AF = mybir.ActivationFunctionType

LAST_RUN = None
_CACHE = {}


def _build_graph(vc):
    nc = bacc.Bacc(None, target_bir_lowering=False)
    x_dram = nc.declare_dram_parameter("x", [B_TILES, P, INPUT_DIM], F32, isOutput=False)
    rhsm_dram = nc.declare_dram_parameter(
        "rhs_main", [2, D_TILES, P, N_CLASSES], BF16, isOutput=False
    )
    rhsc_dram = nc.declare_dram_parameter(
        "rhs_corr", [N_CG, CORR_G, P, D_TILES * N_CLASSES], FP8, isOutput=False
    )
    out_dram = nc.declare_dram_parameter("out", [BL, N_CLASSES], F32, isOutput=True)

    with tile.TileContext(nc) as tc:
        with (
            tc.tile_pool(name="sb", bufs=1) as sb,
            tc.tile_pool(name="ps", bufs=1, space=bass.MemorySpace.PSUM) as ps,
            tc.tile_pool(name="pst", bufs=2, space=bass.MemorySpace.PSUM) as pst,
        ):
            ident = sb.tile([P, P], F32, tag="ident", name="ident")
            make_identity(nc, ident[:])

            # ---- x: one DMA, one fused |x| row-sum, one bf16 cast ----
            x_all = sb.tile([P, B_TILES, INPUT_DIM], F32, tag="x", name="x_all")
            for bt in range(B_TILES):
                (nc.sync if bt % 2 == 0 else nc.scalar).dma_start(
                    out=x_all[:, bt, :], in_=x_dram[bt]
                )
            na_all = sb.tile([P, B_TILES], F32, tag="na", name="na_all")
            nc.vector.tensor_reduce(
                out=na_all[:], in_=x_all[:], axis=mybir.AxisListType.X,
                op=OP.add, apply_absolute_value=True, negate=True,
            )

            # ---- W-side planes (issued early; sync/scalar alternate) ----
            rhsc_sb = []
            for g in range(N_CG):
                t = sb.tile(
                    [P, CORR_G, D_TILES * N_CLASSES], FP8, tag=f"rc{g}", name=f"rc{g}"
                )
                (nc.sync if g % 2 == 0 else nc.scalar).dma_start(
                    out=t[:], in_=rhsc_dram[g].rearrange("j p c -> p j c")
                )
                rhsc_sb.append(t)
            rhsm_sb = []
            for p in range(2):
                t = sb.tile([P, D_TILES, N_CLASSES], BF16, tag=f"rm{p}", name=f"rm{p}")
                (nc.sync if p % 2 == 0 else nc.scalar).dma_start(
                    out=t[:], in_=rhsm_dram[p].rearrange("t p c -> p t c")
                )
                rhsm_sb.append(t)

            # ---- transpose to d-major via PE (bf16) ----
            xT = [
                sb.tile([P, BL], F32, tag=f"xT{t}", name=f"xT{t}")
                for t in range(D_TILES)
            ]
            for bt in range(B_TILES):
                for t in range(D_TILES):
                    tp = pst.tile([P, P], F32, tag="tp", name=f"tp{bt}_{t}")
                    nc.tensor.transpose(
                        tp[:], x_all[:, bt, t * P : (t + 1) * P], ident[:]
                    )
                    nc.vector.tensor_copy(xT[t][:, bt * P : (bt + 1) * P], tp[:])

            # ---- x-side feature planes ----
            pos = [sb.tile([P, BL], BF16, tag=f"pp{t}", name=f"pp{t}") for t in range(D_TILES)]
            neg = [sb.tile([P, BL], BF16, tag=f"nn{t}", name=f"nn{t}") for t in range(D_TILES)]
            for t in range(D_TILES):
                nc.vector.tensor_scalar(
                    out=pos[t][:], in0=xT[t][:], scalar1=0.0, scalar2=None, op0=OP.is_gt
                )
                nc.vector.tensor_scalar(
                    out=neg[t][:], in0=xT[t][:], scalar1=0.0, scalar2=None, op0=OP.is_lt
                )
            corr_pl = []
            for j in range(M_BINS):
                cp = sb.tile([P, D_TILES, BL], FP8, tag=f"cp{j}", name=f"cp{j}")
                for t in range(D_TILES):
                    nc.vector.tensor_scalar(
                        out=cp[:, t, :], in0=xT[t][:],
                        scalar1=0.0, scalar2=float(vc[j]), op0=OP.max, op1=OP.min,
                    )
                corr_pl.append(cp)
            for j in range(M_BINS):
                cn = sb.tile([P, D_TILES, BL], FP8, tag=f"cn{j}", name=f"cn{j}")
                for t in range(D_TILES):
                    nc.vector.tensor_scalar(
                        out=cn[:, t, :], in0=xT[t][:],
                        scalar1=0.0, scalar2=float(-vc[j]), op0=OP.min, op1=OP.max,
                    )
                corr_pl.append(cn)

            # ---- contraction into 4 PSUM banks (one per b-tile) ----
            psum = [
                ps.tile([P, N_CLASSES], F32, tag=f"psum{bt}", name=f"psum{bt}")
                for bt in range(B_TILES)
            ]
            main_pl = [pos, neg]
            for p in range(2):
                for t in range(D_TILES):
                    for bt in range(B_TILES):
                        nc.tensor.matmul(
                            psum[bt][:],
                            main_pl[p][t][:, bt * P : (bt + 1) * P],
                            rhsm_sb[p][:, t, :],
                            start=(p == 0 and t == 0),
                            stop=False,
                        )
            for j in range(N_CORR):
                g, jj = divmod(j, CORR_G)
                rc = rhsc_sb[g][:, jj, :].rearrange("p (t c) -> p t c", t=D_TILES)
                for bt in range(B_TILES):
                    nc.tensor.matmul(
                        psum[bt][:],
                        corr_pl[j][:, :, bt * P : (bt + 1) * P],
                        rc,
                        start=False,
                        stop=(j == N_CORR - 1),
                        perf_mode=mybir.MatmulPerfMode.DoubleRow,
                    )

            # ---- evict (+ negA) and store, two b-tiles per DMA ----
            for g in range(B_TILES // 2):
                o = sb.tile([P, 2, N_CLASSES], F32, tag=f"o{g}", name=f"o{g}")
                for i in range(2):
                    bt = 2 * g + i
                    nc.vector.tensor_scalar(
                        out=o[:, i, :], in0=psum[bt][:],
                        scalar1=na_all[:, bt : bt + 1], scalar2=None, op0=OP.add,
                    )
                (nc.sync if g % 2 == 0 else nc.scalar).dma_start(
                    out=out_dram[2 * g * P : (2 * g + 2) * P, :].rearrange(
                        "(i p) c -> p i c", p=P
                    ),
                    in_=o[:],
                )
    nc.compile()
    return nc


def _host_prep(W, b):
    """Weight preparation: W-side matmul channel planes."""
    C, D = W.shape
    v = np.abs(W)
    vmax = float(v.max()) * 1.000001 + 1e-12
    delta = vmax / M_BINS
    vc = (np.arange(M_BINS) + 0.5) * delta
    bin_idx = np.minimum((v / delta).astype(np.int32), M_BINS - 1)
    vcw = vc[bin_idx].astype(np.float32)
    psi_p = np.where(W > 0, vcw, 0.0).astype(np.float32)
    psi_n = np.where(W < 0, vcw, 0.0).astype(np.float32)
    bias = (b / D)[:, None].astype(np.float32)

    # main channels (bf16): [2, D_TILES, 128, C]
    main = np.stack([(W - 2 * psi_p + bias).T, (-W - 2 * psi_n + bias).T])
    rhs_main = np.ascontiguousarray(main).reshape(2, D_TILES, P, C)
    rhs_main = rhs_main.astype(ml_dtypes.bfloat16)

    # correction channels (fp8 DoubleRow): plane[d, c], d = ko*128 + ki,
    # grouped [N_CG, CORR_G, ki, ko*c] so each partition row is contiguous.
    corr = np.empty((N_CORR, D, C), dtype=np.float32)
    for j in range(M_BINS):
        corr[j] = (2.0 * ((W > 0) & (bin_idx == j))).T
        corr[M_BINS + j] = (-2.0 * ((W < 0) & (bin_idx == j))).T
    corr = corr.reshape(N_CORR, D_TILES, P, C).transpose(0, 2, 1, 3)
    corr = corr.reshape(N_CG, CORR_G, P, D_TILES * C)
    rhs_corr = np.ascontiguousarray(corr).astype(ml_dtypes.float8_e4m3)
    return vc, rhs_main, rhs_corr


def kernel(x, W, b):
    global LAST_RUN
    x = np.ascontiguousarray(np.asarray(x, dtype=np.float32))
    W = np.ascontiguousarray(np.asarray(W, dtype=np.float32))
    b = np.ascontiguousarray(np.asarray(b, dtype=np.float32))
    assert x.shape == (BATCH, INPUT_DIM) and W.shape == (N_CLASSES, INPUT_DIM)

    vc, rhs_main, rhs_corr = _host_prep(W, b)
    key = tuple(np.round(vc, 9).tolist())
    nc = _CACHE.get(key)
    if nc is None:
        nc = _build_graph(vc)
        _CACHE[key] = nc

    in_maps = [
        {
            "x": np.ascontiguousarray(
                x[i * BL : (i + 1) * BL].reshape(B_TILES, P, INPUT_DIM)
            ),
            "rhs_main": rhs_main,
            "rhs_corr": rhs_corr,
        }
        for i in range(N_CORES)
    ]
    LAST_RUN = run_bass_kernel_spmd(
        nc,
        in_maps,
        list(range(N_CORES)),
        trace=bool(int(os.environ.get("KERNEL_TRACE", "0"))),
    )
    out = np.concatenate(
        [np.asarray(LAST_RUN.results[i]["out"]) for i in range(N_CORES)], axis=0
    )
    return out.astype(np.float32)


# revision 9
# speedup vs baseline: 1.1289x; 1.0145x over previous
"""L1-distance classifier (AOClassifier) on 8 TRN2 NeuronCores, data-parallel.

score[b, c] = -sum_d |x[b,d] - W[c,d]| + bias[c]

Exact identity:
    |x - w| = |x| - w*sign(x) + 2*(|w| - |x|)^+ * 1[sign(x) == sign(w)]

The correction term is approximated by quantizing |w| into M_BINS uniform
bins with centers vc_j; per (bin, sign) the x-side factor is clip(x, 0, vc_j)
(resp. clip(x, -vc_j, 0)) and everything collapses into matmul channels:

  score = <P, W - 2*psi_p + b/D> + <N, -W - 2*psi_n + b/D>          (bf16)
        + sum_j <clip(x,0,vc_j), 2*wp_j> - sum_j <clip(x,-vc_j,0), 2*wn_j>
        - sum_d |x[b,d]|                                   (fp32 row-sum)

  P = 1[x>0], N = 1[x<0], psi_p = vc_bin(|w|)*1[w>0], psi_n = vc_bin*1[w<0],
  wp_j = 1[w>0 and bin==j], wn_j = 1[w<0 and bin==j]

Main channels run as bf16 matmuls; the 16 correction channels run as
fp8e4 DoubleRow matmuls (2 weights/PE cell, K=256 per instruction).
W-side planes are weight preparation done once on the host.
Max per-element relative error ~4e-3 vs fp64 reference.
"""

import os

import ml_dtypes
import numpy as np

import concourse.bass as bass
import concourse.mybir as mybir
import concourse.tile as tile
from concourse import bacc
from concourse.bass_utils import run_bass_kernel_spmd
from concourse.masks import make_identity

BATCH, N_CLASSES, INPUT_DIM = 4096, 512, 256
N_CORES = 8
BL = BATCH // N_CORES            # 512 batch rows per core
P = 128                          # SBUF partitions
B_TILES = BL // P                # 4
D_TILES = INPUT_DIM // P         # 2
M_BINS = 8
N_CORR = 2 * M_BINS              # fp8 DoubleRow correction planes
CORR_G = 4                       # correction planes per DMA group
N_CG = N_CORR // CORR_G          # 4 groups

F32 = mybir.dt.float32
BF16 = mybir.dt.bfloat16
FP8 = mybir.dt.float8e4
OP = mybir.AluOpType
AF = mybir.ActivationFunctionType

LAST_RUN = None
_CACHE = {}


def _build_graph(vc):
    nc = bacc.Bacc(None, target_bir_lowering=False)
    x_dram = nc.declare_dram_parameter("x", [B_TILES, P, INPUT_DIM], F32, isOutput=False)
    rhsm_dram = nc.declare_dram_parameter(
        "rhs_main", [2, D_TILES, P, N_CLASSES], BF16, isOutput=False
    )
    rhsc_dram = nc.declare_dram_parameter(
        "rhs_corr", [N_CG, CORR_G, P, D_TILES * N_CLASSES], FP8, isOutput=False
    )
    out_dram = nc.declare_dram_parameter("out", [BL, N_CLASSES], F32, isOutput=True)

    with tile.TileContext(nc) as tc:
        with (
            tc.tile_pool(name="sb", bufs=1) as sb,
            tc.tile_pool(name="ps", bufs=1, space=bass.MemorySpace.PSUM) as ps,
            tc.tile_pool(name="pst", bufs=2, space=bass.MemorySpace.PSUM) as pst,
        ):
            ident = sb.tile([P, P], BF16, tag="ident", name="ident")
            make_identity(nc, ident[:])

            # ---- x: one DMA, one fused |x| row-sum, one bf16 cast ----
            x_all = sb.tile([P, B_TILES, INPUT_DIM], F32, tag="x", name="x_all")
            H = INPUT_DIM // 2
            for bt in range(B_TILES):
                for h in range(2):
                    (nc.sync if (2 * bt + h) % 2 == 0 else nc.scalar).dma_start(
                        out=x_all[:, bt, h * H : (h + 1) * H],
                        in_=x_dram[bt, :, h * H : (h + 1) * H],
                    )
            xb_all = sb.tile([P, B_TILES, INPUT_DIM], BF16, tag="xb", name="xb_all")
            for bt in range(B_TILES):
                nc.vector.tensor_copy(xb_all[:, bt, :], x_all[:, bt, :])
            na_all = sb.tile([P, B_TILES], F32, tag="na", name="na_all")
            nc.vector.tensor_reduce(
                out=na_all[:], in_=x_all[:], axis=mybir.AxisListType.X,
                op=OP.add, apply_absolute_value=True, negate=True,
            )

            # ---- W-side planes (issued early; sync/scalar alternate) ----
            rhsc_sb = []
            for g in range(N_CG):
                t = sb.tile(
                    [P, CORR_G, D_TILES * N_CLASSES], FP8, tag=f"rc{g}", name=f"rc{g}"
                )
                (nc.sync if g % 2 == 0 else nc.scalar).dma_start(
                    out=t[:], in_=rhsc_dram[g].rearrange("j p c -> p j c")
                )
                rhsc_sb.append(t)
            rhsm_sb = []
            for p in range(2):
                t = sb.tile([P, D_TILES, N_CLASSES], BF16, tag=f"rm{p}", name=f"rm{p}")
                (nc.sync if p % 2 == 0 else nc.scalar).dma_start(
                    out=t[:], in_=rhsm_dram[p].rearrange("t p c -> p t c")
                )
                rhsm_sb.append(t)

            # ---- transpose to d-major via PE (bf16) ----
            xT = [
                sb.tile([P, BL], BF16, tag=f"xT{t}", name=f"xT{t}")
                for t in range(D_TILES)
            ]
            for bt in range(B_TILES):
                for t in range(D_TILES):
                    tp = pst.tile([P, P], BF16, tag="tp", name=f"tp{bt}_{t}")
                    nc.tensor.transpose(
                        tp[:], xb_all[:, bt, t * P : (t + 1) * P], ident[:]
                    )
                    nc.vector.tensor_copy(xT[t][:, bt * P : (bt + 1) * P], tp[:])

            # ---- x-side feature planes ----
            pos = [sb.tile([P, BL], BF16, tag=f"pp{t}", name=f"pp{t}") for t in range(D_TILES)]
            neg = [sb.tile([P, BL], BF16, tag=f"nn{t}", name=f"nn{t}") for t in range(D_TILES)]
            for t in range(D_TILES):
                nc.vector.tensor_scalar(
                    out=pos[t][:], in0=xT[t][:], scalar1=0.0, scalar2=None, op0=OP.is_gt
                )
                nc.vector.tensor_scalar(
                    out=neg[t][:], in0=xT[t][:], scalar1=0.0, scalar2=None, op0=OP.is_lt
                )
            corr_pl = []
            for j in range(M_BINS):
                cp = sb.tile([P, D_TILES, BL], FP8, tag=f"cp{j}", name=f"cp{j}")
                for t in range(D_TILES):
                    nc.vector.tensor_scalar(
                        out=cp[:, t, :], in0=xT[t][:],
                        scalar1=0.0, scalar2=float(vc[j]), op0=OP.max, op1=OP.min,
                    )
                corr_pl.append(cp)
            for j in range(M_BINS):
                cn = sb.tile([P, D_TILES, BL], FP8, tag=f"cn{j}", name=f"cn{j}")
                for t in range(D_TILES):
                    nc.gpsimd.tensor_scalar(
                        out=cn[:, t, :], in0=xT[t][:],
                        scalar1=0.0, scalar2=float(-vc[j]), op0=OP.min, op1=OP.max,
                    )
                corr_pl.append(cn)

            # ---- contraction into 4 PSUM banks (one per b-tile) ----
            psum = [
                ps.tile([P, N_CLASSES], F32, tag=f"psum{bt}", name=f"psum{bt}")
                for bt in range(B_TILES)
            ]
            main_pl = [pos, neg]
            for p in range(2):
                for t in range(D_TILES):
                    for bt in range(B_TILES):
                        nc.tensor.matmul(
                            psum[bt][:],
                            main_pl[p][t][:, bt * P : (bt + 1) * P],
                            rhsm_sb[p][:, t, :],
                            start=(p == 0 and t == 0),
                            stop=False,
                        )
            for j in range(N_CORR):
                g, jj = divmod(j, CORR_G)
                rc = rhsc_sb[g][:, jj, :].rearrange("p (t c) -> p t c", t=D_TILES)
                for bt in range(B_TILES):
                    nc.tensor.matmul(
                        psum[bt][:],
                        corr_pl[j][:, :, bt * P : (bt + 1) * P],
                        rc,
                        start=False,
                        stop=(j == N_CORR - 1),
                        perf_mode=mybir.MatmulPerfMode.DoubleRow,
                    )

            # ---- evict (+ negA) and store, two b-tiles per DMA ----
            for g in range(B_TILES // 2):
                o = sb.tile([P, 2, N_CLASSES], F32, tag=f"o{g}", name=f"o{g}")
                for i in range(2):
                    bt = 2 * g + i
                    nc.vector.tensor_scalar(
                        out=o[:, i, :], in0=psum[bt][:],
                        scalar1=na_all[:, bt : bt + 1], scalar2=None, op0=OP.add,
                    )
                (nc.sync if g % 2 == 0 else nc.scalar).dma_start(
                    out=out_dram[2 * g * P : (2 * g + 2) * P, :].rearrange(
                        "(i p) c -> p i c", p=P
                    ),
                    in_=o[:],
                )
    nc.compile()
    return nc


def _host_prep(W, b):
    """Weight preparation: W-side matmul channel planes."""
    C, D = W.shape
    v = np.abs(W)
    vmax = float(v.max()) * 1.000001 + 1e-12
    delta = vmax / M_BINS
    vc = (np.arange(M_BINS) + 0.5) * delta
    bin_idx = np.minimum((v / delta).astype(np.int32), M_BINS - 1)
    vcw = vc[bin_idx].astype(np.float32)
    psi_p = np.where(W > 0, vcw, 0.0).astype(np.float32)
    psi_n = np.where(W < 0, vcw, 0.0).astype(np.float32)
    bias = (b / D)[:, None].astype(np.float32)

    # main channels (bf16): [2, D_TILES, 128, C]
    main = np.stack([(W - 2 * psi_p + bias).T, (-W - 2 * psi_n + bias).T])
    rhs_main = np.ascontiguousarray(main).reshape(2, D_TILES, P, C)
    rhs_main = rhs_main.astype(ml_dtypes.bfloat16)

    # correction channels (fp8 DoubleRow): plane[d, c], d = ko*128 + ki,
    # grouped [N_CG, CORR_G, ki, ko*c] so each partition row is contiguous.
    corr = np.empty((N_CORR, D, C), dtype=np.float32)
    for j in range(M_BINS):
        corr[j] = (2.0 * ((W > 0) & (bin_idx == j))).T
        corr[M_BINS + j] = (-2.0 * ((W < 0) & (bin_idx == j))).T
    corr = corr.reshape(N_CORR, D_TILES, P, C).transpose(0, 2, 1, 3)
    corr = corr.reshape(N_CG, CORR_G, P, D_TILES * C)
    rhs_corr = np.ascontiguousarray(corr).astype(ml_dtypes.float8_e4m3)
    return vc, rhs_main, rhs_corr


def kernel(x, W, b):
    global LAST_RUN
    x = np.ascontiguousarray(np.asarray(x, dtype=np.float32))
    W = np.ascontiguousarray(np.asarray(W, dtype=np.float32))
    b = np.ascontiguousarray(np.asarray(b, dtype=np.float32))
    assert x.shape == (BATCH, INPUT_DIM) and W.shape == (N_CLASSES, INPUT_DIM)

    vc, rhs_main, rhs_corr = _host_prep(W, b)
    key = tuple(np.round(vc, 9).tolist())
    nc = _CACHE.get(key)
    if nc is None:
        nc = _build_graph(vc)
        _CACHE[key] = nc

    in_maps = [
        {
            "x": np.ascontiguousarray(
                x[i * BL : (i + 1) * BL].reshape(B_TILES, P, INPUT_DIM)
            ),
            "rhs_main": rhs_main,
            "rhs_corr": rhs_corr,
        }
        for i in range(N_CORES)
    ]
    LAST_RUN = run_bass_kernel_spmd(
        nc,
        in_maps,
        list(range(N_CORES)),
        trace=bool(int(os.environ.get("KERNEL_TRACE", "0"))),
    )
    out = np.concatenate(
        [np.asarray(LAST_RUN.results[i]["out"]) for i in range(N_CORES)], axis=0
    )
    return out.astype(np.float32)
